# revision 17
# baseline (speedup 1.0000x reference)
"""Trainium2 Bass kernel for nn_ChunkedAttention (B=4, T=4096, D=1024, H=16, dh=64).

Sharding: 8 cores = 4 batches x 2 head-groups (8 heads each).
Each core computes, for its batch b and head-group g:
  Q^T/K^T/V projections (fp32r matmuls), causal flash-style attention
  (scores kept transposed [k, q] so softmax sums come from a ones-column
  in the AV matmul), and the partial out-projection over its 512 head
  dims. Host sums the two partials per batch.

All matmuls run as float32r (FP22 truncation, full PE rate at N>=256).
"""

import os
import sys

import numpy as np

for _p in ("/opt/trn_rl_repo",):
    if _p not in sys.path and os.path.isdir(_p):
        sys.path.insert(0, _p)

import concourse.bass as bass
import concourse.mybir as mybir
import concourse.tile as tile
from concourse.bacc import Bacc
from concourse.bass_utils import run_bass_kernel_spmd

F32 = mybir.dt.float32
F32R = mybir.dt.float32r
EXP = mybir.ActivationFunctionType.Exp
MULT = mybir.AluOpType.mult

B, T, D = 4, 4096, 1024
HG = 512          # head-group width per core (8 heads x 64)
NH, DH = 8, 64    # heads per core, head dim
NPAIR = 4         # head pairs per core
QG = 512          # query-group width
NQG = T // QG     # 8
NKT = T // 128    # 32 k-tiles
NTC = T // 512    # 8 T-chunks in projection phase
NDC = D // 128    # 8 d_model chunks
SCALE = 1.0 / np.sqrt(DH)  # 0.125


def _r(ap):
    return ap.bitcast(F32R)


def build_nc(debug=None):
    if debug is None:
        debug = os.environ.get("KDEBUG", "0") == "1"
    nc = Bacc()
    x_d = nc.dram_tensor("x", [T, D], F32, kind="ExternalInput")
    wqT_d = nc.dram_tensor("wqT", [D, HG], F32, kind="ExternalInput")
    wkT_d = nc.dram_tensor("wkT", [D, HG], F32, kind="ExternalInput")
    wvT_d = nc.dram_tensor("wvT", [D, HG], F32, kind="ExternalInput")
    woT_d = nc.dram_tensor("woT", [HG, D], F32, kind="ExternalInput")
    tri_d = nc.dram_tensor("tri", [128, 128], F32, kind="ExternalInput")
    id_d = nc.dram_tensor("ident", [128, 128], F32, kind="ExternalInput")
    y_d = nc.dram_tensor("y", [T, D], F32, kind="ExternalOutput")
    qt_d = nc.dram_tensor("qt_stage", [NPAIR, 128, T], F32R)
    if debug:
        kt_dump = nc.dram_tensor("kt_dump", [128, NPAIR, T], F32R, kind="ExternalOutput")
        v_dump = nc.dram_tensor("v_dump", [128, NKT, NH, DH + 1], F32R, kind="ExternalOutput")
        q_dump = nc.dram_tensor("q_dump", [NPAIR, 128, T], F32R, kind="ExternalOutput")
        rs_dump = nc.dram_tensor("rs_dump", [NQG, NPAIR, 2, QG], F32, kind="ExternalOutput")
        rb_dump = nc.dram_tensor("rb_dump", [NPAIR, 2, 64, QG], F32, kind="ExternalOutput")
        mrg_dump = nc.dram_tensor("mrg_dump", [NQG, 128, NPAIR, QG], F32R, kind="ExternalOutput")
        e_dump = nc.dram_tensor("e_dump", [NPAIR, 128, 2, QG], F32R, kind="ExternalOutput")

    with tile.TileContext(nc) as tc:
        with (
            tc.tile_pool(name="const", bufs=1) as pconst,
            tc.tile_pool(name="psA", bufs=2, space="PSUM") as psA,
            tc.tile_pool(name="psB", bufs=4, space="PSUM") as psB,
        ):
            kt_sb = pconst.tile([128, NPAIR, T], F32R, tag="kt")
            v_sb = pconst.tile([128, NKT, NH, DH + 1], F32R, tag="v")
            tri_sb = pconst.tile([128, 128], F32R, tag="tri")
            id_sb = pconst.tile([128, 128], F32, tag="ident")
            nc.sync.dma_start(tri_sb[:], tri_d[:].bitcast(F32R))
            nc.sync.dma_start(id_sb[:], id_d[:])
            nc.gpsimd.memset(v_sb[:, :, :, DH : DH + 1].bitcast(F32), 1.0)

            # ---------- phase 1: x^T + Q^T/K^T/V projections ----------
            with (
                tc.tile_pool(name="ph1", bufs=1) as p1,
                tc.tile_pool(name="xin", bufs=2) as pxin,
            ):
                wq_sb = p1.tile([128, NDC, HG], F32R, tag="wq")
                wk_sb = p1.tile([128, NDC, HG], F32R, tag="wk")
                wv_sb = p1.tile([128, NDC, HG], F32R, tag="wv")
                nc.sync.dma_start(
                    wq_sb[:], wqT_d.bitcast(F32R).rearrange("(dc p) h -> p dc h", p=128)
                )
                nc.sync.dma_start(
                    wk_sb[:], wkT_d.bitcast(F32R).rearrange("(dc p) h -> p dc h", p=128)
                )
                nc.sync.dma_start(
                    wv_sb[:], wvT_d.bitcast(F32R).rearrange("(dc p) h -> p dc h", p=128)
                )
                x_r = x_d.rearrange("(nt p) d -> nt p d", p=128)
                for tcn in range(NTC):
                    xt_sb = p1.tile([128, NDC, 512], F32R, tag="xt")
                    for ts in range(4):
                        xin = pxin.tile([128, D], F32, tag="xin")
                        nc.sync.dma_start(xin[:], x_r[tcn * 4 + ts])
                        for dc in range(NDC):
                            tp = psA.tile([128, 128], F32, tag="big")
                            nc.tensor.transpose(
                                tp[:],
                                xin[:, dc * 128 : (dc + 1) * 128],
                                id_sb[:],
                            )
                            nc.scalar.copy(
                                xt_sb[:, dc, ts * 128 : (ts + 1) * 128], tp[:]
                            )
                    # Q^T -> DRAM staging
                    for hp in range(NPAIR):
                        pq = psB.tile([128, 512], F32, tag="one")
                        for dc in range(NDC):
                            nc.tensor.matmul(
                                pq[:],
                                wq_sb[:, dc, hp * 128 : (hp + 1) * 128],
                                xt_sb[:, dc, :],
                                start=(dc == 0),
                                stop=(dc == NDC - 1),
                            )
                        qst = pxin.tile([128, 512], F32R, tag="qst")
                        nc.vector.tensor_copy(qst[:], pq[:])
                        nc.sync.dma_start(
                            qt_d[hp, :, tcn * 512 : (tcn + 1) * 512], qst[:]
                        )
                    # K^T -> resident SBUF
                    for hp in range(NPAIR):
                        pk = psB.tile([128, 512], F32, tag="one")
                        for dc in range(NDC):
                            nc.tensor.matmul(
                                pk[:],
                                wk_sb[:, dc, hp * 128 : (hp + 1) * 128],
                                xt_sb[:, dc, :],
                                start=(dc == 0),
                                stop=(dc == NDC - 1),
                            )
                        nc.vector.tensor_copy(
                            kt_sb[:, hp, tcn * 512 : (tcn + 1) * 512], pk[:]
                        )
                    # V -> resident SBUF (natural [k, h, dh] layout)
                    for ts in range(4):
                        pv = psB.tile([128, 512], F32, tag="one")
                        for dc in range(NDC):
                            nc.tensor.matmul(
                                pv[:],
                                xt_sb[:, dc, ts * 128 : (ts + 1) * 128],
                                wv_sb[:, dc, :],
                                start=(dc == 0),
                                stop=(dc == NDC - 1),
                            )
                        ktg = tcn * 4 + ts
                        nc.vector.tensor_copy(
                            v_sb[:, ktg, :, 0:DH],
                            pv.rearrange("p (h d) -> p h d", h=NH),
                        )

            if debug:
                nc.sync.dma_start(kt_dump[:], kt_sb[:])
                nc.sync.dma_start(v_dump[:], v_sb[:])
                nc.sync.dma_start(q_dump[:], qt_d[:])

            # ---------- phase 2: attention + out-projection ----------
            with (
                tc.tile_pool(name="ph2", bufs=1) as p2,
                tc.tile_pool(name="ph2b", bufs=2) as p2b,
                tc.tile_pool(name="ph2c", bufs=3) as p2c,
            ):
                wo_sb = p2.tile([128, NPAIR, D], F32R, tag="wo")
                nc.sync.dma_start(
                    wo_sb[:], woT_d.bitcast(F32R).rearrange("(hp p) e -> p hp e", p=128)
                )
                for qg in range(NQG):
                    mrg = p2b.tile([128, NPAIR, QG], F32R, tag="mrg")
                    ktmax = 4 * (qg + 1)
                    for hp in range(NPAIR):
                        qt_t = p2b.tile([128, QG], F32R, tag="qt")
                        nc.sync.dma_start(
                            qt_t[:], qt_d[hp, :, qg * QG : (qg + 1) * QG]
                        )
                        av = [
                            psB.tile([DH + 1, QG], F32, tag="one", name=f"av{j}")
                            for j in range(2)
                        ]
                        for kt in range(ktmax):
                            diag = kt - 4 * qg
                            dlt = 128 * diag if diag >= 0 else 0
                            s_t = psA.tile([128, 2, 512], F32, tag="big")
                            for j in range(2):
                                nc.tensor.matmul(
                                    s_t[:, j, dlt:],
                                    kt_sb[
                                        64 * j : 64 * (j + 1),
                                        hp,
                                        kt * 128 : (kt + 1) * 128,
                                    ],
                                    qt_t[64 * j : 64 * (j + 1), dlt:],
                                    start=True,
                                    stop=True,
                                    tile_position=(64 * j, 0),
                                )
                            e_t = p2c.tile([128, 2, QG], F32R, tag="exps")
                            nc.scalar.activation(
                                e_t[:, :, dlt:], s_t[:, :, dlt:], EXP, scale=SCALE
                            )
                            if debug and qg == 0 and kt == 0:
                                nc.sync.dma_start(e_dump[hp], e_t[:])
                            if diag >= 0:
                                for j in range(2):
                                    nc.vector.tensor_tensor(
                                        e_t[:, j, dlt : dlt + 128],
                                        e_t[:, j, dlt : dlt + 128],
                                        tri_sb[:],
                                        MULT,
                                    )
                            for j in range(2):
                                nc.tensor.matmul(
                                    av[j][:, dlt:],
                                    v_sb[:, kt, 2 * hp + j, :],
                                    e_t[:, j, dlt:],
                                    start=(kt == 0),
                                    stop=(kt == ktmax - 1),
                                )
                        for j in range(2):
                            rs_t = p2c.tile([DH + 1, QG], F32, tag="rs")
                            nc.vector.reciprocal(
                                rs_t[DH : DH + 1, :], av[j][DH : DH + 1, :]
                            )
                            rs0 = p2c.tile([1, QG], F32, tag="rs")
                            nc.sync.dma_start(rs0[:], rs_t[DH : DH + 1, :])
                            rb_t = p2c.tile([64, QG], F32, tag="rb")
                            nc.gpsimd.partition_broadcast(rb_t[:], rs0[:])
                            if debug:
                                nc.sync.dma_start(
                                    rs_dump[qg, hp, j : j + 1, :],
                                    rs_t[DH : DH + 1, :],
                                )
                                if qg == NQG - 1:
                                    nc.sync.dma_start(rb_dump[hp, j], rb_t[:])
                            if j == 0:
                                nc.vector.tensor_tensor(
                                    mrg[0:64, hp, :],
                                    av[j][0:DH, :],
                                    rb_t[:],
                                    MULT,
                                )
                            else:
                                odt = p2c.tile([64, QG], F32R, tag="odt")
                                nc.vector.tensor_tensor(
                                    odt[:], av[j][0:DH, :], rb_t[:], MULT
                                )
                                nc.sync.dma_start(mrg[64:128, hp, :], odt[:])
                    if debug:
                        nc.sync.dma_start(mrg_dump[qg], mrg[:])
                    for qc in range(4):
                        for half in range(2):
                            op = psB.tile([128, 512], F32, tag="one")
                            for hp in range(NPAIR):
                                nc.tensor.matmul(
                                    op[:],
                                    mrg[:, hp, qc * 128 : (qc + 1) * 128],
                                    wo_sb[:, hp, half * 512 : (half + 1) * 512],
                                    start=(hp == 0),
                                    stop=(hp == NPAIR - 1),
                                )
                            yt = p2c.tile([128, 512], F32, tag="yt")
                            nc.vector.tensor_copy(yt[:], op[:])
                            nc.sync.dma_start(
                                y_d[
                                    qg * QG + qc * 128 : qg * QG + (qc + 1) * 128,
                                    half * 512 : (half + 1) * 512,
                                ],
                                yt[:],
                            )
    nc.compile()
    return nc


_NC_CACHE = None


def _get_nc():
    global _NC_CACHE
    if _NC_CACHE is None:
        _NC_CACHE = build_nc()
    return _NC_CACHE


def make_in_maps(x, Wq, Wk, Wv, Wo):
    x = np.asarray(x, dtype=np.float32)
    Wq = np.asarray(Wq, dtype=np.float32)
    Wk = np.asarray(Wk, dtype=np.float32)
    Wv = np.asarray(Wv, dtype=np.float32)
    Wo = np.asarray(Wo, dtype=np.float32)
    tri = np.triu(np.ones((128, 128), dtype=np.float32))
    ident = np.eye(128, dtype=np.float32)
    in_maps = []
    for c in range(8):
        b, g = divmod(c, 2)
        rows = slice(HG * g, HG * (g + 1))
        in_maps.append(
            {
                "x": np.ascontiguousarray(x[b]),
                "wqT": np.ascontiguousarray(Wq[rows].T),
                "wkT": np.ascontiguousarray(Wk[rows].T),
                "wvT": np.ascontiguousarray(Wv[rows].T),
                "woT": np.ascontiguousarray(Wo[:, rows].T),
                "tri": tri,
                "ident": ident,
            }
        )
    return in_maps


def run(x, Wq, Wk, Wv, Wo, trace=False, **spmd_kwargs):
    nc = _get_nc()
    in_maps = make_in_maps(x, Wq, Wk, Wv, Wo)
    res = run_bass_kernel_spmd(
        nc, in_maps, core_ids=list(range(8)), trace=trace, **spmd_kwargs
    )
    parts = [np.asarray(r["y"], dtype=np.float32) for r in res.results]
    y = np.stack([parts[2 * b] + parts[2 * b + 1] for b in range(B)])
    return y, res


def kernel(x, Wq, Wk, Wv, Wo):
    y, _ = run(x, Wq, Wk, Wv, Wo, trace=False)
    return y


# revision 18
# speedup vs baseline: 196.0716x; 196.0716x over previous
"""Trainium2 Bass kernel for nn_ChunkedAttention (B=4, T=4096, D=1024, H=16, dh=64).

Sharding: 8 cores = 4 batches x 2 head-groups (8 heads each).
Each core computes, for its batch b and head-group g:
  Q^T/K^T/V projections (fp32r matmuls), causal flash-style attention
  (scores kept transposed [k, q] so softmax sums come from a ones-column
  in the AV matmul), and the partial out-projection over its 512 head
  dims. Host sums the two partials per batch.

All matmuls run as float32r (FP22 truncation, full PE rate at N>=256).
"""

import os
import sys

import numpy as np

for _p in ("/opt/trn_rl_repo",):
    if _p not in sys.path and os.path.isdir(_p):
        sys.path.insert(0, _p)

import concourse.bass as bass
import concourse.mybir as mybir
import concourse.tile as tile
from concourse.bacc import Bacc
from concourse.bass_utils import run_bass_kernel_spmd

F32 = mybir.dt.float32
F32R = mybir.dt.float32r
EXP = mybir.ActivationFunctionType.Exp
MULT = mybir.AluOpType.mult

B, T, D = 4, 4096, 1024
HG = 512          # head-group width per core (8 heads x 64)
NH, DH = 8, 64    # heads per core, head dim
NPAIR = 4         # head pairs per core
QG = 512          # query-group width
NQG = T // QG     # 8
NKT = T // 128    # 32 k-tiles
NTC = T // 512    # 8 T-chunks in projection phase
NDC = D // 128    # 8 d_model chunks
SCALE = 1.0 / np.sqrt(DH)  # 0.125


def _r(ap):
    return ap.bitcast(F32R)


def build_nc(debug=None):
    if debug is None:
        debug = os.environ.get("KDEBUG", "0") == "1"
    nc = Bacc()
    x_d = nc.dram_tensor("x", [T, D], F32, kind="ExternalInput")
    wqT_d = nc.dram_tensor("wqT", [D, HG], F32, kind="ExternalInput")
    wkT_d = nc.dram_tensor("wkT", [D, HG], F32, kind="ExternalInput")
    wvT_d = nc.dram_tensor("wvT", [D, HG], F32, kind="ExternalInput")
    woT_d = nc.dram_tensor("woT", [HG, D], F32, kind="ExternalInput")
    tri_d = nc.dram_tensor("tri", [128, 128], F32, kind="ExternalInput")
    id_d = nc.dram_tensor("ident", [128, 128], F32, kind="ExternalInput")
    y_d = nc.dram_tensor("y", [T, D], F32, kind="ExternalOutput")
    qt_d = nc.dram_tensor("qt_stage", [NPAIR, 128, T], F32R)
    if debug:
        kt_dump = nc.dram_tensor("kt_dump", [128, NPAIR, T], F32R, kind="ExternalOutput")
        v_dump = nc.dram_tensor("v_dump", [128, NKT, NH, DH + 1], F32R, kind="ExternalOutput")
        q_dump = nc.dram_tensor("q_dump", [NPAIR, 128, T], F32R, kind="ExternalOutput")
        rs_dump = nc.dram_tensor("rs_dump", [NQG, NPAIR, 2, QG], F32, kind="ExternalOutput")
        rb_dump = nc.dram_tensor("rb_dump", [NPAIR, 2, 64, QG], F32, kind="ExternalOutput")
        mrg_dump = nc.dram_tensor("mrg_dump", [NQG, 128, NPAIR, QG], F32R, kind="ExternalOutput")
        e_dump = nc.dram_tensor("e_dump", [NPAIR, 128, 2, QG], F32R, kind="ExternalOutput")

    with tile.TileContext(nc) as tc:
        with (
            tc.tile_pool(name="const", bufs=1) as pconst,
            tc.tile_pool(name="psA", bufs=2, space="PSUM") as psA,
            tc.tile_pool(name="psB", bufs=2, space="PSUM") as psB,
            tc.tile_pool(name="psC", bufs=2, space="PSUM") as psC,
        ):
            kt_sb = pconst.tile([128, NPAIR, T], F32R, tag="kt")
            v_sb = pconst.tile([128, NKT, NH, DH + 1], F32R, tag="v")
            tri_sb = pconst.tile([128, 128], F32R, tag="tri")
            id_sb = pconst.tile([128, 128], F32, tag="ident")
            nc.sync.dma_start(tri_sb[:], tri_d[:].bitcast(F32R))
            nc.sync.dma_start(id_sb[:], id_d[:])
            nc.gpsimd.memset(v_sb[:, :, :, DH : DH + 1].bitcast(F32), 1.0)

            # ---------- phase 1: x^T + Q^T/K^T/V projections ----------
            with (
                tc.tile_pool(name="ph1", bufs=1) as p1,
                tc.tile_pool(name="pxt", bufs=2) as pxt,
                tc.tile_pool(name="xin", bufs=2) as pxin,
            ):
                wq_sb = p1.tile([128, NDC, HG], F32R, tag="wq")
                wk_sb = p1.tile([128, NDC, HG], F32R, tag="wk")
                wv_sb = p1.tile([128, NDC, HG], F32R, tag="wv")
                nc.sync.dma_start(
                    wq_sb[:], wqT_d.bitcast(F32R).rearrange("(dc p) h -> p dc h", p=128)
                )
                nc.sync.dma_start(
                    wk_sb[:], wkT_d.bitcast(F32R).rearrange("(dc p) h -> p dc h", p=128)
                )
                nc.sync.dma_start(
                    wv_sb[:], wvT_d.bitcast(F32R).rearrange("(dc p) h -> p dc h", p=128)
                )
                x_r = x_d.rearrange("(nt p) d -> nt p d", p=128)
                for tcn in range(NTC):
                    xt_sb = pxt.tile([128, NDC, 512], F32R, tag="xt")
                    for ts in range(4):
                        xin = pxin.tile([128, D], F32, tag="xin")
                        nc.sync.dma_start(xin[:], x_r[tcn * 4 + ts])
                        for dc in range(NDC):
                            tp = psA.tile([128, 128], F32, tag="big")
                            nc.tensor.transpose(
                                tp[:],
                                xin[:, dc * 128 : (dc + 1) * 128],
                                id_sb[:],
                            )
                            nc.scalar.copy(
                                xt_sb[:, dc, ts * 128 : (ts + 1) * 128], tp[:]
                            )
                    # Q^T -> DRAM staging
                    for hp in range(NPAIR):
                        pq = (psB if hp % 2 == 0 else psC).tile(
                            [128, 512], F32, tag="one", name="pq"
                        )
                        for dc in range(NDC):
                            nc.tensor.matmul(
                                pq[:],
                                wq_sb[:, dc, hp * 128 : (hp + 1) * 128],
                                xt_sb[:, dc, :],
                                start=(dc == 0),
                                stop=(dc == NDC - 1),
                            )
                        qst = pxin.tile([128, 512], F32R, tag="qst")
                        nc.vector.tensor_copy(qst[:], pq[:])
                        nc.sync.dma_start(
                            qt_d[hp, :, tcn * 512 : (tcn + 1) * 512], qst[:]
                        )
                    # K^T -> resident SBUF
                    for hp in range(NPAIR):
                        pk = (psB if hp % 2 == 0 else psC).tile(
                            [128, 512], F32, tag="one", name="pk"
                        )
                        for dc in range(NDC):
                            nc.tensor.matmul(
                                pk[:],
                                wk_sb[:, dc, hp * 128 : (hp + 1) * 128],
                                xt_sb[:, dc, :],
                                start=(dc == 0),
                                stop=(dc == NDC - 1),
                            )
                        nc.vector.tensor_copy(
                            kt_sb[:, hp, tcn * 512 : (tcn + 1) * 512], pk[:]
                        )
                    # V -> resident SBUF (natural [k, h, dh] layout)
                    for ts in range(4):
                        pv = (psB if ts % 2 == 0 else psC).tile(
                            [128, 512], F32, tag="one", name="pv"
                        )
                        for dc in range(NDC):
                            nc.tensor.matmul(
                                pv[:],
                                xt_sb[:, dc, ts * 128 : (ts + 1) * 128],
                                wv_sb[:, dc, :],
                                start=(dc == 0),
                                stop=(dc == NDC - 1),
                            )
                        ktg = tcn * 4 + ts
                        nc.vector.tensor_copy(
                            v_sb[:, ktg, :, 0:DH],
                            pv.rearrange("p (h d) -> p h d", h=NH),
                        )

            if debug:
                nc.sync.dma_start(kt_dump[:], kt_sb[:])
                nc.sync.dma_start(v_dump[:], v_sb[:])
                nc.sync.dma_start(q_dump[:], qt_d[:])

            # ---------- phase 2: attention + out-projection ----------
            with (
                tc.tile_pool(name="ph2", bufs=1) as p2,
                tc.tile_pool(name="ph2b", bufs=2) as p2b,
                tc.tile_pool(name="ph2c", bufs=3) as p2c,
            ):
                wo_sb = p2.tile([128, NPAIR, D], F32R, tag="wo")
                nc.sync.dma_start(
                    wo_sb[:], woT_d.bitcast(F32R).rearrange("(hp p) e -> p hp e", p=128)
                )
                for qg in range(NQG):
                    mrg = p2b.tile([128, NPAIR, QG], F32R, tag="mrg")
                    ktmax = 4 * (qg + 1)
                    for hp in range(NPAIR):
                        qt_t = p2b.tile([128, QG], F32R, tag="qt")
                        nc.sync.dma_start(
                            qt_t[:], qt_d[hp, :, qg * QG : (qg + 1) * QG]
                        )
                        av = [
                            psB.tile([DH + 1, QG], F32, tag="one", name=f"av{j}")
                            for j in range(2)
                        ]
                        for kt in range(ktmax):
                            diag = kt - 4 * qg
                            dlt = 128 * diag if diag >= 0 else 0
                            s_t = psA.tile([128, 2, 512], F32, tag="big")
                            for j in range(2):
                                nc.tensor.matmul(
                                    s_t[:, j, dlt:],
                                    kt_sb[
                                        64 * j : 64 * (j + 1),
                                        hp,
                                        kt * 128 : (kt + 1) * 128,
                                    ],
                                    qt_t[64 * j : 64 * (j + 1), dlt:],
                                    start=True,
                                    stop=True,
                                    tile_position=(64 * j, 0),
                                )
                            e_t = p2c.tile([128, 2, QG], F32R, tag="exps")
                            nc.scalar.activation(
                                e_t[:, :, dlt:], s_t[:, :, dlt:], EXP, scale=SCALE
                            )
                            if debug and qg == 0 and kt == 0:
                                nc.sync.dma_start(e_dump[hp], e_t[:])
                            if diag >= 0:
                                for j in range(2):
                                    nc.vector.tensor_tensor(
                                        e_t[:, j, dlt : dlt + 128],
                                        e_t[:, j, dlt : dlt + 128],
                                        tri_sb[:],
                                        MULT,
                                    )
                            for j in range(2):
                                nc.tensor.matmul(
                                    av[j][:, dlt:],
                                    v_sb[:, kt, 2 * hp + j, :],
                                    e_t[:, j, dlt:],
                                    start=(kt == 0),
                                    stop=(kt == ktmax - 1),
                                )
                        for j in range(2):
                            rs_t = p2c.tile([DH + 1, QG], F32, tag="rs")
                            nc.vector.reciprocal(
                                rs_t[DH : DH + 1, :], av[j][DH : DH + 1, :]
                            )
                            rs0 = p2c.tile([1, QG], F32, tag="rs")
                            nc.sync.dma_start(rs0[:], rs_t[DH : DH + 1, :])
                            rb_t = p2c.tile([64, QG], F32, tag="rb")
                            nc.gpsimd.partition_broadcast(rb_t[:], rs0[:])
                            if debug:
                                nc.sync.dma_start(
                                    rs_dump[qg, hp, j : j + 1, :],
                                    rs_t[DH : DH + 1, :],
                                )
                                if qg == NQG - 1:
                                    nc.sync.dma_start(rb_dump[hp, j], rb_t[:])
                            if j == 0:
                                nc.vector.tensor_tensor(
                                    mrg[0:64, hp, :],
                                    av[j][0:DH, :],
                                    rb_t[:],
                                    MULT,
                                )
                            else:
                                odt = p2c.tile([64, QG], F32R, tag="odt")
                                nc.vector.tensor_tensor(
                                    odt[:], av[j][0:DH, :], rb_t[:], MULT
                                )
                                nc.sync.dma_start(mrg[64:128, hp, :], odt[:])
                    if debug:
                        nc.sync.dma_start(mrg_dump[qg], mrg[:])
                    for qc in range(4):
                        for half in range(2):
                            op = psC.tile([128, 512], F32, tag="one")
                            for hp in range(NPAIR):
                                nc.tensor.matmul(
                                    op[:],
                                    mrg[:, hp, qc * 128 : (qc + 1) * 128],
                                    wo_sb[:, hp, half * 512 : (half + 1) * 512],
                                    start=(hp == 0),
                                    stop=(hp == NPAIR - 1),
                                )
                            yt = p2c.tile([128, 512], F32, tag="yt")
                            nc.vector.tensor_copy(yt[:], op[:])
                            nc.sync.dma_start(
                                y_d[
                                    qg * QG + qc * 128 : qg * QG + (qc + 1) * 128,
                                    half * 512 : (half + 1) * 512,
                                ],
                                yt[:],
                            )
    nc.compile()
    return nc


_NC_CACHE = None


def _get_nc():
    global _NC_CACHE
    if _NC_CACHE is None:
        _NC_CACHE = build_nc()
    return _NC_CACHE


def make_in_maps(x, Wq, Wk, Wv, Wo):
    x = np.asarray(x, dtype=np.float32)
    Wq = np.asarray(Wq, dtype=np.float32)
    Wk = np.asarray(Wk, dtype=np.float32)
    Wv = np.asarray(Wv, dtype=np.float32)
    Wo = np.asarray(Wo, dtype=np.float32)
    tri = np.triu(np.ones((128, 128), dtype=np.float32))
    ident = np.eye(128, dtype=np.float32)
    in_maps = []
    for c in range(8):
        b, g = divmod(c, 2)
        rows = slice(HG * g, HG * (g + 1))
        in_maps.append(
            {
                "x": np.ascontiguousarray(x[b]),
                "wqT": np.ascontiguousarray(Wq[rows].T),
                "wkT": np.ascontiguousarray(Wk[rows].T),
                "wvT": np.ascontiguousarray(Wv[rows].T),
                "woT": np.ascontiguousarray(Wo[:, rows].T),
                "tri": tri,
                "ident": ident,
            }
        )
    return in_maps


def run(x, Wq, Wk, Wv, Wo, trace=False, **spmd_kwargs):
    nc = _get_nc()
    in_maps = make_in_maps(x, Wq, Wk, Wv, Wo)
    res = run_bass_kernel_spmd(
        nc, in_maps, core_ids=list(range(8)), trace=trace, **spmd_kwargs
    )
    parts = [np.asarray(r["y"], dtype=np.float32) for r in res.results]
    y = np.stack([parts[2 * b] + parts[2 * b + 1] for b in range(B)])
    return y, res


def kernel(x, Wq, Wk, Wv, Wo):
    y, _ = run(x, Wq, Wk, Wv, Wo, trace=False)
    return y


# revision 30
# speedup vs baseline: 16604.5450x; 84.6861x over previous
"""Trainium2 Bass kernel for nn_ChunkedAttention (B=4, T=4096, D=1024, H=16, dh=64).

Sharding: 8 cores = 4 batches x 2 head-groups (8 heads each).
Each core computes, for its batch b and head-group g:
  Q^T/K^T/V projections (fp32r matmuls), causal flash-style attention
  (scores kept transposed [k, q] so softmax sums come from a ones-column
  in the AV matmul), and the partial out-projection over its 512 head
  dims. Host sums the two partials per batch.

All matmuls run as float32r (FP22 truncation, full PE rate at N>=256).
"""

import os
import sys

import numpy as np

for _p in ("/opt/trn_rl_repo",):
    if _p not in sys.path and os.path.isdir(_p):
        sys.path.insert(0, _p)

import concourse.bass as bass
import concourse.mybir as mybir
import concourse.tile as tile
from concourse.bacc import Bacc
from concourse.bass_utils import run_bass_kernel_spmd

F32 = mybir.dt.float32
F32R = mybir.dt.float32r
EXP = mybir.ActivationFunctionType.Exp
MULT = mybir.AluOpType.mult

B, T, D = 4, 4096, 1024
HG = 512          # head-group width per core (8 heads x 64)
NH, DH = 8, 64    # heads per core, head dim
NPAIR = 4         # head pairs per core
QG = 512          # query-group width
NQG = T // QG     # 8
NKT = T // 128    # 32 k-tiles
NTC = T // 512    # 8 T-chunks in projection phase
NDC = D // 128    # 8 d_model chunks
SCALE = 1.0 / np.sqrt(DH)  # 0.125


def _r(ap):
    return ap.bitcast(F32R)


def build_nc(debug=None):
    if debug is None:
        debug = os.environ.get("KDEBUG", "0") == "1"
    nc = Bacc()
    x_d = nc.dram_tensor("x", [T, D], F32, kind="ExternalInput")
    wqT_d = nc.dram_tensor("wqT", [D, HG], F32, kind="ExternalInput")
    wkT_d = nc.dram_tensor("wkT", [D, HG], F32, kind="ExternalInput")
    wvT_d = nc.dram_tensor("wvT", [D, HG], F32, kind="ExternalInput")
    woT_d = nc.dram_tensor("woT", [HG, D], F32, kind="ExternalInput")
    tri_d = nc.dram_tensor("tri", [128, 128], F32, kind="ExternalInput")
    id_d = nc.dram_tensor("ident", [128, 128], F32, kind="ExternalInput")
    y_d = nc.dram_tensor("y", [T, D], F32, kind="ExternalOutput")
    qt_d = nc.dram_tensor("qt_stage", [NPAIR, 128, T], F32R)
    if debug:
        kt_dump = nc.dram_tensor("kt_dump", [128, NPAIR, T], F32R, kind="ExternalOutput")
        v_dump = nc.dram_tensor("v_dump", [128, NKT, NH, DH + 1], F32R, kind="ExternalOutput")
        q_dump = nc.dram_tensor("q_dump", [NPAIR, 128, T], F32R, kind="ExternalOutput")
        rs_dump = nc.dram_tensor("rs_dump", [NQG, NPAIR, 2, QG], F32, kind="ExternalOutput")
        rb_dump = nc.dram_tensor("rb_dump", [NPAIR, 2, 64, QG], F32, kind="ExternalOutput")
        mrg_dump = nc.dram_tensor("mrg_dump", [NQG, 128, NPAIR, QG], F32R, kind="ExternalOutput")
        e_dump = nc.dram_tensor("e_dump", [NPAIR, 128, 2, QG], F32R, kind="ExternalOutput")

    with tile.TileContext(nc) as tc:
        with (
            tc.tile_pool(name="const", bufs=1) as pconst,
            tc.tile_pool(name="psA", bufs=2, space="PSUM") as psA,
            tc.tile_pool(name="psB", bufs=4, space="PSUM") as psB,
        ):
            kt_sb = pconst.tile([128, NPAIR, T], F32R, tag="kt")
            v_sb = pconst.tile([128, NKT, NH, DH + 1], F32R, tag="v")
            tri_sb = pconst.tile([128, 128], F32R, tag="tri")
            id_sb = pconst.tile([128, 128], F32, tag="ident")
            nc.sync.dma_start(tri_sb[:], tri_d[:].bitcast(F32R))
            nc.sync.dma_start(id_sb[:], id_d[:])
            nc.gpsimd.memset(v_sb[:, :, :, DH : DH + 1].bitcast(F32), 1.0)

            # ---------- phase 1: x^T + Q^T/K^T/V projections ----------
            with (
                tc.tile_pool(name="ph1", bufs=1) as p1,
                tc.tile_pool(name="pxt", bufs=1) as pxt,
                tc.tile_pool(name="xin", bufs=2) as pxin,
            ):
                wq_sb = p1.tile([128, NDC, HG], F32R, tag="wq")
                wk_sb = p1.tile([128, NDC, HG], F32R, tag="wk")
                wv_sb = p1.tile([128, NDC, HG], F32R, tag="wv")
                nc.sync.dma_start(
                    wq_sb[:], wqT_d.bitcast(F32R).rearrange("(dc p) h -> p dc h", p=128)
                )
                nc.sync.dma_start(
                    wk_sb[:], wkT_d.bitcast(F32R).rearrange("(dc p) h -> p dc h", p=128)
                )
                nc.sync.dma_start(
                    wv_sb[:], wvT_d.bitcast(F32R).rearrange("(dc p) h -> p dc h", p=128)
                )
                x_r = x_d.rearrange("(nt p) d -> nt p d", p=128)
                for tcn in range(NTC):
                    xt_dc = [
                        pxt.tile([128, 512], F32R, tag=f"xt{dc}", name=f"xt{dc}")
                        for dc in range(NDC)
                    ]
                    for ts in range(4):
                        xin = pxin.tile([128, D], F32, tag="xin")
                        nc.sync.dma_start(xin[:], x_r[tcn * 4 + ts])
                        for dc in range(NDC):
                            tp = psA.tile([128, 128], F32, tag="big")
                            nc.tensor.transpose(
                                tp[:],
                                xin[:, dc * 128 : (dc + 1) * 128],
                                id_sb[:],
                            )
                            nc.vector.tensor_copy(
                                xt_dc[dc][:, ts * 128 : (ts + 1) * 128], tp[:]
                            )
                    # Q^T -> DRAM staging
                    for hp in range(NPAIR):
                        pq = psB.tile([128, 512], F32, tag="one", name="pq")
                        for dc in range(NDC):
                            nc.tensor.matmul(
                                pq[:],
                                wq_sb[:, dc, hp * 128 : (hp + 1) * 128],
                                xt_dc[dc][:],
                                start=(dc == 0),
                                stop=(dc == NDC - 1),
                            )
                        qst = pxin.tile([128, 512], F32R, tag="qst")
                        nc.vector.tensor_copy(qst[:], pq[:])
                        nc.sync.dma_start(
                            qt_d[hp, :, tcn * 512 : (tcn + 1) * 512], qst[:]
                        )
                    # K^T -> resident SBUF
                    for hp in range(NPAIR):
                        pk = psB.tile([128, 512], F32, tag="one", name="pk")
                        for dc in range(NDC):
                            nc.tensor.matmul(
                                pk[:],
                                wk_sb[:, dc, hp * 128 : (hp + 1) * 128],
                                xt_dc[dc][:],
                                start=(dc == 0),
                                stop=(dc == NDC - 1),
                            )
                        nc.vector.tensor_copy(
                            kt_sb[:, hp, tcn * 512 : (tcn + 1) * 512], pk[:]
                        )
                    # V -> resident SBUF (natural [k, h, dh] layout)
                    for ts in range(4):
                        pv = psB.tile([128, 512], F32, tag="one", name="pv")
                        for dc in range(NDC):
                            nc.tensor.matmul(
                                pv[:],
                                xt_dc[dc][:, ts * 128 : (ts + 1) * 128],
                                wv_sb[:, dc, :],
                                start=(dc == 0),
                                stop=(dc == NDC - 1),
                            )
                        ktg = tcn * 4 + ts
                        nc.vector.tensor_copy(
                            v_sb[:, ktg, :, 0:DH],
                            pv.rearrange("p (h d) -> p h d", h=NH),
                        )

            if debug:
                nc.sync.dma_start(kt_dump[:], kt_sb[:])
                nc.sync.dma_start(v_dump[:], v_sb[:])
                nc.sync.dma_start(q_dump[:], qt_d[:])

            # ---------- phase 2: attention + out-projection ----------
            with (
                tc.tile_pool(name="ph2", bufs=1) as p2,
                tc.tile_pool(name="ph2b", bufs=2) as p2b,
                tc.tile_pool(name="ph2c", bufs=4) as p2c,
                tc.tile_pool(name="ph2d", bufs=3) as p2d,
            ):
                wo_sb = p2.tile([128, NPAIR, D], F32R, tag="wo")
                nc.sync.dma_start(
                    wo_sb[:], woT_d.bitcast(F32R).rearrange("(hp p) e -> p hp e", p=128)
                )
                for qg in range(NQG):
                    mrg = p2b.tile([128, NPAIR, QG], F32R, tag="mrg")
                    ktmax = 4 * (qg + 1)
                    for hp in range(NPAIR):
                        qt_t = p2b.tile([128, QG], F32R, tag="qt")
                        nc.sync.dma_start(
                            qt_t[:], qt_d[hp, :, qg * QG : (qg + 1) * QG]
                        )
                        av = [
                            psB.tile([DH + 1, QG], F32, tag="one", name=f"av{j}")
                            for j in range(2)
                        ]
                        for kt in range(ktmax):
                            diag = kt - 4 * qg
                            dlt = 128 * diag if diag >= 0 else 0
                            s_t = psA.tile([128, 2, 512], F32, tag="big")
                            for j in range(2):
                                nc.tensor.matmul(
                                    s_t[:, j, dlt:],
                                    kt_sb[
                                        64 * j : 64 * (j + 1),
                                        hp,
                                        kt * 128 : (kt + 1) * 128,
                                    ],
                                    qt_t[64 * j : 64 * (j + 1), dlt:],
                                    start=True,
                                    stop=True,
                                    tile_position=(64 * j, 0),
                                )
                            e_t = p2c.tile([128, 2, QG], F32R, tag="exps")
                            nc.scalar.activation(
                                e_t[:, :, dlt:], s_t[:, :, dlt:], EXP, scale=SCALE
                            )
                            if debug and qg == 0 and kt == 0:
                                nc.sync.dma_start(e_dump[hp], e_t[:])
                            if diag >= 0:
                                for j in range(2):
                                    nc.vector.tensor_tensor(
                                        e_t[:, j, dlt : dlt + 128],
                                        e_t[:, j, dlt : dlt + 128],
                                        tri_sb[:],
                                        MULT,
                                    )
                            for j in range(2):
                                nc.tensor.matmul(
                                    av[j][:, dlt:],
                                    v_sb[:, kt, 2 * hp + j, :],
                                    e_t[:, j, dlt:],
                                    start=(kt == 0),
                                    stop=(kt == ktmax - 1),
                                )
                        for j in range(2):
                            rs_t = p2d.tile([DH + 1, QG], F32, tag="rs")
                            nc.vector.reciprocal(
                                rs_t[DH : DH + 1, :], av[j][DH : DH + 1, :]
                            )
                            rs0 = p2d.tile([1, QG], F32, tag="rs")
                            nc.sync.dma_start(rs0[:], rs_t[DH : DH + 1, :])
                            rb_t = p2d.tile([64, QG], F32, tag="rb")
                            nc.gpsimd.partition_broadcast(rb_t[:], rs0[:])
                            if debug:
                                nc.sync.dma_start(
                                    rs_dump[qg, hp, j : j + 1, :],
                                    rs_t[DH : DH + 1, :],
                                )
                                if qg == NQG - 1:
                                    nc.sync.dma_start(rb_dump[hp, j], rb_t[:])
                            if j == 0:
                                nc.vector.tensor_tensor(
                                    mrg[0:64, hp, :],
                                    av[j][0:DH, :],
                                    rb_t[:],
                                    MULT,
                                )
                            else:
                                odt = p2d.tile([64, QG], F32R, tag="odt")
                                nc.vector.tensor_tensor(
                                    odt[:], av[j][0:DH, :], rb_t[:], MULT
                                )
                                nc.sync.dma_start(mrg[64:128, hp, :], odt[:])
                    if debug:
                        nc.sync.dma_start(mrg_dump[qg], mrg[:])
                    for qc in range(4):
                        for half in range(2):
                            op = psB.tile([128, 512], F32, tag="one")
                            for hp in range(NPAIR):
                                nc.tensor.matmul(
                                    op[:],
                                    mrg[:, hp, qc * 128 : (qc + 1) * 128],
                                    wo_sb[:, hp, half * 512 : (half + 1) * 512],
                                    start=(hp == 0),
                                    stop=(hp == NPAIR - 1),
                                )
                            yt = p2d.tile([128, 512], F32, tag="yt")
                            nc.vector.tensor_copy(yt[:], op[:])
                            nc.sync.dma_start(
                                y_d[
                                    qg * QG + qc * 128 : qg * QG + (qc + 1) * 128,
                                    half * 512 : (half + 1) * 512,
                                ],
                                yt[:],
                            )
    nc.compile()
    return nc


_NC_CACHE = None


def _get_nc():
    global _NC_CACHE
    if _NC_CACHE is None:
        _NC_CACHE = build_nc()
    return _NC_CACHE


def make_in_maps(x, Wq, Wk, Wv, Wo):
    x = np.asarray(x, dtype=np.float32)
    Wq = np.asarray(Wq, dtype=np.float32)
    Wk = np.asarray(Wk, dtype=np.float32)
    Wv = np.asarray(Wv, dtype=np.float32)
    Wo = np.asarray(Wo, dtype=np.float32)
    tri = np.triu(np.ones((128, 128), dtype=np.float32))
    ident = np.eye(128, dtype=np.float32)
    in_maps = []
    for c in range(8):
        b, g = divmod(c, 2)
        rows = slice(HG * g, HG * (g + 1))
        in_maps.append(
            {
                "x": np.ascontiguousarray(x[b]),
                "wqT": np.ascontiguousarray(Wq[rows].T),
                "wkT": np.ascontiguousarray(Wk[rows].T),
                "wvT": np.ascontiguousarray(Wv[rows].T),
                "woT": np.ascontiguousarray(Wo[:, rows].T),
                "tri": tri,
                "ident": ident,
            }
        )
    return in_maps


def run(x, Wq, Wk, Wv, Wo, trace=False, **spmd_kwargs):
    nc = _get_nc()
    in_maps = make_in_maps(x, Wq, Wk, Wv, Wo)
    res = run_bass_kernel_spmd(
        nc, in_maps, core_ids=list(range(8)), trace=trace, **spmd_kwargs
    )
    parts = [np.asarray(r["y"], dtype=np.float32) for r in res.results]
    y = np.stack([parts[2 * b] + parts[2 * b + 1] for b in range(B)])
    return y, res


def kernel(x, Wq, Wk, Wv, Wo):
    y, _ = run(x, Wq, Wk, Wv, Wo, trace=False)
    return y
